# revision 1
# baseline (speedup 1.0000x reference)
"""Trainium2 Bass kernel for nn_Attention_35923106463893.

Multi-head attention block:
    qkv = (weight[:, :, None] * (x @ W_qkv)) -> split q,k,v over 12 heads
    A = softmax(q k^T / sqrt(64));  out = (A v) reshaped @ W_msa + b_msa

Sharding: pure data-parallel over batch B=8 -> one batch element per
NeuronCore, no collectives. Host pre-transposes x[b] so the device never
transposes activations; per-head attention is computed in "transposed"
layout (keys on partitions) so softmax denominators come from an appended
ones-column of V and normalization uses a partition broadcast — no
on-chip transposes of the attention matrix at all. exp() skips the
max-subtraction: scores for this problem's distribution are in [-7, 7].

All matmuls run in fp32r (full-rate fp32, ~2e-4 relative error).
The emission order software-pipelines the in-order PE stream: later qk
projection chunks and the V projection are interleaved into the
ACT(exp)-gated attention stream so no engine starves.
"""

from contextlib import ExitStack

import numpy as np

import concourse.bass as bass
import concourse.mybir as mybir
import concourse.tile as tile
from concourse import bacc
from concourse.bass import ts
from concourse.bass_utils import run_bass_kernel_spmd

B, N, D, H = 8, 1024, 768, 12
HD = D // H          # 64
SCALE = HD ** -0.5   # 0.125
KC = D // 128        # 6 contraction chunks
MC_QK = (2 * D) // 128  # 12 row-chunks of [q;k]^T
NT = N // 128        # 8 token chunks
NC2 = N // 512       # 2 moving chunks

F32 = mybir.dt.float32
F32R = mybir.dt.float32r
AF = mybir.ActivationFunctionType

_CACHE: dict = {}


def _run(gen):
    for _ in gen:
        pass


def _emit(tc, repeat=1):
    nc = tc.nc
    xt_d = nc.dram_tensor("xt", [D, N], F32R, kind="ExternalInput").ap()
    w_d = nc.dram_tensor("w", [1, N], F32, kind="ExternalInput").ap()
    wqk_d = nc.dram_tensor("wqk", [D, 2 * D], F32R, kind="ExternalInput").ap()
    wv_d = nc.dram_tensor("wv", [D, D], F32R, kind="ExternalInput").ap()
    wm_d = nc.dram_tensor("wmsa", [D, D], F32R, kind="ExternalInput").ap()
    bm_d = nc.dram_tensor("bmsa", [D], F32, kind="ExternalInput").ap()
    y_d = nc.dram_tensor("yt", [D, N], F32, kind="ExternalOutput").ap()

    for _rep in range(repeat):
        _emit_body(tc, xt_d, w_d, wqk_d, wv_d, wm_d, bm_d, y_d)


def _emit_body(tc, xt_d, w_d, wqk_d, wv_d, wm_d, bm_d, y_d):
    nc = tc.nc
    with ExitStack() as s1:
        const = s1.enter_context(tc.tile_pool(name="const", bufs=1))
        pwm = s1.enter_context(tc.tile_pool(name="pwm", bufs=1))
        pwv = s1.enter_context(tc.tile_pool(name="pwv", bufs=1))
        pqk = s1.enter_context(tc.tile_pool(name="pqk", bufs=1))
        pv = s1.enter_context(tc.tile_pool(name="pv", bufs=1))
        pot = s1.enter_context(tc.tile_pool(name="pot", bufs=1))
        pxt = s1.enter_context(tc.tile_pool(name="pxt", bufs=1))
        pwqs = s1.enter_context(tc.tile_pool(name="pwqs", bufs=2))
        pe_ = s1.enter_context(tc.tile_pool(name="pe", bufs=5))
        pdn = s1.enter_context(tc.tile_pool(name="pdn", bufs=2))
        pbc = s1.enter_context(tc.tile_pool(name="pbc", bufs=1))
        pfin = s1.enter_context(tc.tile_pool(name="pfin", bufs=2))
        psA = s1.enter_context(tc.tile_pool(name="psA", bufs=2, space="PSUM"))
        psB = s1.enter_context(tc.tile_pool(name="psB", bufs=2, space="PSUM"))

        # ---- startup-critical loads: first qk weights + x^T ----
        # one strided DMA gathers all six [128,128] stationary chunks of a
        # qk column-block into a single [128, 768] tile
        wqk3 = wqk_d.rearrange("(c p) e -> p c e", p=128)

        def load_wq(m, eng=nc.sync):
            t = pwqs.tile([128, KC * 128], F32R, tag="wqs", name="wqs")
            eng.dma_start(
                t[:].rearrange("p (c e) -> p c e", e=128),
                wqk3[:, :, ts(m, 128)],
            )
            return t

        wq_pre = load_wq(0, eng=nc.scalar)
        wq_pre2 = load_wq(KC, eng=nc.scalar)
        xtt = [pxt.tile([128, N], F32R, tag=f"xt{c}", name=f"xt{c}") for c in range(KC)]
        # x^T loads split across both HWDGE queues
        for c in range(KC):
            nc.sync.dma_start(xtt[c][:, 0:512], xt_d[ts(c, 128), 0:512])
            nc.scalar.dma_start(xtt[c][:, 512:1024], xt_d[ts(c, 128), 512:1024])
        onescol_f = const.tile([128, HD], F32, tag="onescol_f")
        nc.vector.memset(onescol_f[:], 1.0)
        onescol = const.tile([128, HD], F32R, tag="onescol")
        nc.vector.tensor_copy(onescol[:], onescol_f[:])

        # ---- PE warm-up: the HAM clock gate holds PE at 1.2 GHz until
        # ~3.4us of sustained activity; run junk matmuls on already-resident
        # tiles during the otherwise-idle DMA wait so real work starts warm
        psj = psA.tile([128, 512], F32, tag="psA", name="psj")
        for _ in range(12):
            nc.tensor.matmul(
                psj[0:HD, 0:HD], onescol[:], onescol[:], start=True, stop=True
            )
        for _ in range(6):
            nc.tensor.matmul(
                psj[0:HD, :], onescol[:], xtt[0][:, 0:512], start=True, stop=True
            )
        for _ in range(6):
            nc.tensor.matmul(
                psj[0:HD, :], onescol[:], wq_pre[:, 0:512], start=True, stop=True
            )

        # non-critical loads queue behind x^T
        w_row = pdn.tile([1, N], F32, tag="dn", name="w_row")
        nc.sync.dma_start(w_row[:], w_d[:])
        bias = const.tile([128, KC], F32, tag="bias")
        for c in range(KC):
            nc.sync.dma_start(
                bias[:, c : c + 1],
                bm_d[ts(c, 128)].rearrange("(p o) -> p o", o=1),
            )
        wvt = [pwv.tile([128, D], F32R, tag=f"wv{c}", name=f"wv{c}") for c in range(KC)]

        # Token gating is folded into the PSUM->SBUF copies downstream of the
        # projections (not applied to x^T), keeping it off the startup path:
        # qk^T rows are scaled by wb (free-dim broadcast of w), V rows by
        # wcol (per-partition scalar, w transposed to [128, NT]).
        wb = const.tile([128, N], F32, tag="wb")
        nc.gpsimd.partition_broadcast(wb[:], w_row[:])
        wcol = const.tile([128, NT], F32, tag="wcol")
        nc.sync.dma_start(wcol[:], w_d[0, :].rearrange("(r p) -> p r", p=128))

        qkt = [pqk.tile([128, N], F32R, tag=f"qk{m}", name=f"qk{m}") for m in range(MC_QK)]
        vt = [
            pv.tile([128, H * (HD + 1)], F32R, tag=f"v{r}", name=f"v{r}")
            for r in range(NT)
        ]
        ott = [pot.tile([128, N], F32R, tag=f"ot{c}", name=f"ot{c}") for c in range(KC)]
        wmt = [pwm.tile([128, D], F32R, tag=f"wm{c}", name=f"wm{c}") for c in range(KC)]

        def gen_qk(m, pre=None):
            """qk^T chunk m: [128, N] = W_qkv[:, 128m:...]^T @ xg. Yields per c."""
            wq_m = pre if pre is not None else load_wq(m)
            ps = psB.tile([128, N], F32, tag="psB", name="psB")
            for c in range(KC):
                for j in range(NC2):
                    nc.tensor.matmul(
                        ps[:, ts(j, 512)],
                        wq_m[:, ts(c, 128)],
                        xtt[c][:, ts(j, 512)],
                        start=(c == 0),
                        stop=(c == KC - 1),
                    )
                yield
            # extra yield: the copy is deferred past the surrounding head's
            # normalization chain so it doesn't delay the po release on DVE
            yield
            nc.vector.tensor_mul(qkt[m][:], ps[:], wb[:])

        def gen_v():
            """V in natural layout + ones column per head. Yields per r."""
            for r in range(NT):
                pvp = psB.tile([128, D], F32, tag="psB", name="psB")
                for off, wd in ((0, 512), (512, 256)):
                    for c in range(KC):
                        nc.tensor.matmul(
                            pvp[:, off : off + wd],
                            xtt[c][:, ts(r, 128)],
                            wvt[c][:, off : off + wd],
                            start=(c == 0),
                            stop=(c == KC - 1),
                        )
                v3 = vt[r][:].rearrange("p (h e) -> p h e", e=HD + 1)
                nc.vector.tensor_copy(
                    v3[:, :, HD : HD + 1],
                    onescol[:, 0:H].rearrange("p (h o) -> p h o", o=1),
                )
                nc.vector.tensor_scalar_mul(
                    v3[:, :, 0:HD],
                    pvp[:].rearrange("p (h e) -> p h e", e=HD),
                    wcol[:, r : r + 1],
                )
                yield

        def do_o(h, r, e, po):
            # accumulate [v; 1]^T @ E^T -> rows 0:64 = unnormalized
            # attention out (transposed), row 64 = softmax denominator
            for j in range(NC2):
                nc.tensor.matmul(
                    po[:, ts(j, 512)],
                    vt[r][:, h * (HD + 1) : (h + 1) * (HD + 1)],
                    e[:, ts(j, 512)],
                    start=(r == 0),
                    stop=(r == NT - 1),
                )

        def norm_head(h, po):
            # normalize: custom-DVE ops misread PSUM at a partition offset on
            # HW, so stage the denominator row through SBUF partition 0;
            # broadcast+multiply pipelined in halves across GPSIMD/DVE
            dnr = pdn.tile([1, N], F32, tag="dn", name="dnr")
            nc.vector.tensor_copy(dnr[:], po[HD : HD + 1, :])
            dn = pdn.tile([1, N], F32, tag="dn", name="dn")
            nc.vector.reciprocal_approx_fast(dn[:], dnr[:])
            bc = pbc.tile([HD, N], F32, tag="bc", name="bc")
            orow = ott[h // 2][HD * (h % 2) : HD * (h % 2) + HD, :]
            for j in range(NC2):
                nc.gpsimd.partition_broadcast(bc[:, ts(j, 512)], dn[:, ts(j, 512)])
                nc.vector.tensor_mul(
                    orow[:, ts(j, 512)],
                    po[0:HD, ts(j, 512)],
                    bc[:, ts(j, 512)],
                )

        def gen_attn(h):
            """Attention head h. Yields per r-chunk (8 steps), then normalizes.

            The PE stream is software-pipelined one stage: S(r+1) is emitted
            before O'(r) so O' never waits on exp(r) in-stream."""
            qt, qr = qkt[h // 2], HD * (h % 2)
            kt, kr = qkt[KC + h // 2], HD * (h % 2)
            po = psB.tile([HD + 1, N], F32, tag="psB", name="psB")
            pend = []  # O' accumulation steps, software-pipelined 2 deep:
            # the S->exp->O' latency chain (~2.3us) exceeds one step of PE
            # work (~1.7us), so O'(r) issues two steps after S(r)
            for r in range(NT):
                ps = psA.tile([128, N], F32, tag="psA", name="psA")
                # S^T chunk: [keys 128, queries 1024]
                for j in range(NC2):
                    nc.tensor.matmul(
                        ps[:, ts(j, 512)],
                        kt[kr : kr + HD, ts(r, 128)],
                        qt[qr : qr + HD, ts(j, 512)],
                        start=True,
                        stop=True,
                    )
                e = pe_.tile([128, N], F32R, tag="e", name="e")
                nc.scalar.activation(e[:], ps[:], AF.Exp, scale=SCALE)
                if len(pend) == 4:
                    do_o(h, *pend.pop(0), po)
                pend.append((r, e))
                yield
            for r_, e_ in pend:
                do_o(h, r_, e_, po)
            norm_head(h, po)

        def interleave(main, filler, skip=0, ratio=1.5):
            """Exhaust `main`; after main step i >= skip, advance `filler`
            by ~ratio steps (fractional accumulator)."""
            owed = 0.0
            for i, _ in enumerate(main):
                if i >= skip:
                    owed += ratio
                    while owed >= 1.0:
                        next(filler, None)
                        owed -= 1.0
            _run(filler)

        def chain(*gens):
            for g in gens:
                yield from g

        # ---- schedule ----
        # qk chunks for heads 0/1 first, then V interleaved with head 0;
        # remaining qk chunks ride one-per-head inside the exp-gated
        # attention streams (head h's chunk finishes >=1 head before its
        # consumer), so at most one qk PSUM accumulator is live at a time.
        _run(gen_qk(0, pre=wq_pre))
        _run(gen_qk(KC, pre=wq_pre2))
        # V-projection weights load behind the qk weight streams
        for c in range(KC):
            eng = nc.sync if c % 2 == 0 else nc.scalar
            eng.dma_start(wvt[c][:], wv_d[ts(c, 128), :])
        # output projection pieces; chunk 0's first 5 k-steps double as PE
        # filler inside head 10 (its accumulation group stays open in psB)
        def gen_proj(c, ps, k_from, k_to):
            for k in range(k_from, k_to):
                for j in range(NC2):
                    nc.tensor.matmul(
                        ps[:, ts(j, 512)],
                        wmt[k][:, ts(c, 128)],
                        ott[k][:, ts(j, 512)],
                        start=(k == 0),
                        stop=(k == KC - 1),
                    )
                yield

        def finish_proj(c, ps, k_from):
            _run(gen_proj(c, ps, k_from, KC))
            fin = pfin.tile([128, N], F32, tag="fin", name="fin")
            nc.scalar.activation(fin[:], ps[:], AF.Identity, bias=bias[:, c : c + 1])
            nc.sync.dma_start(y_d[ts(c, 128), :], fin[:])

        interleave(gen_v(), gen_attn(0), ratio=1.0)
        filler_map = {
            1: [1, KC + 1, 2],
            2: [KC + 2],
            3: [3],
            4: [KC + 3],
            5: [4],
            6: [KC + 4],
            7: [5],
            8: [KC + 5],
        }
        for h in range(1, H):
            chunks = filler_map.get(h, [])
            if chunks:
                interleave(
                    gen_attn(h),
                    chain(*[gen_qk(m) for m in chunks]),
                    skip=2,
                    ratio=len(chunks) * KC / (NT - 2),
                )
            else:
                _run(gen_attn(h))
            if h == 2:
                # W_msa loads overlap the attention phase
                for c in range(KC):
                    nc.sync.dma_start(wmt[c][:], wm_d[ts(c, 128), :])

        # ---- output projection + bias ----
        for c in range(KC):
            ps = psA.tile([128, N], F32, tag="psA", name="psA")
            finish_proj(c, ps, 0)


def _build(repeat=1):
    key = ("nc", repeat)
    if key not in _CACHE:
        nc = bacc.Bacc("TRN2", target_bir_lowering=False, debug=False, num_devices=B)
        with tile.TileContext(nc) as tc:
            _emit(tc, repeat=repeat)
        nc.compile()
        _CACHE[key] = nc
    return _CACHE[key]


def kernel(x, weight, W_qkv, W_msa, b_msa):
    nc = _build()
    x = np.asarray(x, dtype=np.float32)
    weight = np.asarray(weight, dtype=np.float32)
    W_qkv = np.asarray(W_qkv, dtype=np.float32)
    wqk = np.ascontiguousarray(W_qkv[:, : 2 * D])
    wv = np.ascontiguousarray(W_qkv[:, 2 * D :])
    in_maps = []
    for b in range(B):
        in_maps.append(
            {
                "xt": np.ascontiguousarray(x[b].T),
                "w": np.ascontiguousarray(weight[b : b + 1]),
                "wqk": wqk,
                "wv": wv,
                "wmsa": np.asarray(W_msa, dtype=np.float32),
                "bmsa": np.asarray(b_msa, dtype=np.float32),
            }
        )
    res = run_bass_kernel_spmd(nc, in_maps, list(range(B)))
    out = np.stack([res.results[b]["yt"].T for b in range(B)], axis=0)
    return np.ascontiguousarray(out.astype(np.float32))



# revision 40
# speedup vs baseline: 1.0649x; 1.0649x over previous
"""Trainium2 Bass kernel for nn_Attention_35923106463893.

Multi-head attention block:
    qkv = (weight[:, :, None] * (x @ W_qkv)) -> split q,k,v over 12 heads
    A = softmax(q k^T / sqrt(64));  out = (A v) reshaped @ W_msa + b_msa

Sharding: pure data-parallel over batch B=8 -> one batch element per
NeuronCore, no collectives. Host pre-transposes x[b] and pre-casts all
inputs to bf16 (halves DMA bytes; rel-err ~3e-3 vs the 2e-2 gate).

Key structure:
  - gating folded into x^T once (diag(w) (x W) = (diag(w) x) W)
  - attention per head in transposed layout (keys on partitions); V gets
    a leading ones-column so O' row 0 accumulates the softmax
    denominator at PSUM partition 0, where the fast DVE reciprocal can
    read it in place; normalization via Pool partition-broadcast.
  - exp() skips max-subtraction: logits here are in [-7, 7].
  - DMAs are consolidated into few large transfers: the HWDGE queues
    have a ~1.6us per-descriptor-set cadence, so many small DMAs are
    issue-bound, not bandwidth-bound.
  - the in-order PE stream is software-pipelined: qk chunks for later
    heads ride inside the exp-gated attention streams as fillers; the
    output projection defers every ott[5]-dependent k-step past head
    11's normalization and drains through alternating ACT/DVE readouts
    into one bf16 staging buffer written out by 4 grouped DMAs.
"""

from contextlib import ExitStack

import numpy as np

import concourse.bass as bass
import concourse.mybir as mybir
import concourse.tile as tile
from concourse import bacc
from concourse.bass import ts
from concourse.bass_utils import run_bass_kernel_spmd

B, N, D, H = 8, 1024, 768, 12
HD = D // H          # 64
SCALE = HD ** -0.5   # 0.125
KC = D // 128        # 6 contraction chunks
MC_QK = (2 * D) // 128  # 12 row-chunks of [q;k]^T
NT = N // 128        # 8 token chunks
NC2 = N // 512       # 2 moving chunks

F32 = mybir.dt.float32
F32R = mybir.dt.float32r
BF16 = mybir.dt.bfloat16
AF = mybir.ActivationFunctionType

_CACHE: dict = {}
LABELS: dict = {}


def _lab(inst, label):
    try:
        LABELS[inst.ins.name] = label
    except Exception:
        pass
    return inst


def _run(gen):
    for _ in gen:
        pass


def _emit(tc, repeat=1):
    nc = tc.nc
    # all inputs prepacked on the host into the exact SBUF layouts so
    # every load is row-contiguous (the DMA model pays 2x for runs <512B)
    xt_d = nc.dram_tensor("xt", [128, KC * N], BF16, kind="ExternalInput").ap()
    w_d = nc.dram_tensor("w", [128, N], BF16, kind="ExternalInput").ap()
    wqk_d = nc.dram_tensor("wqk", [KC * 128, 2 * D], BF16, kind="ExternalInput").ap()
    wv_d = nc.dram_tensor("wv", [128, KC * D], BF16, kind="ExternalInput").ap()
    wm_d = nc.dram_tensor("wmsa", [128, KC * D], BF16, kind="ExternalInput").ap()
    bm_d = nc.dram_tensor("bmsa", [D], F32, kind="ExternalInput").ap()
    y_d = nc.dram_tensor("yt", [D, N], BF16, kind="ExternalOutput").ap()

    for _rep in range(repeat):
        _emit_body(tc, xt_d, w_d, wqk_d, wv_d, wm_d, bm_d, y_d)


def _emit_body(tc, xt_d, w_d, wqk_d, wv_d, wm_d, bm_d, y_d):
    nc = tc.nc
    with ExitStack() as s1:
        const = s1.enter_context(tc.tile_pool(name="const", bufs=1))
        pbig = s1.enter_context(tc.tile_pool(name="pbig", bufs=1))
        pqk = s1.enter_context(tc.tile_pool(name="pqk", bufs=1))
        pv = s1.enter_context(tc.tile_pool(name="pv", bufs=1))
        pot = s1.enter_context(tc.tile_pool(name="pot", bufs=1))
        pe_ = s1.enter_context(tc.tile_pool(name="pe", bufs=8))
        pdn = s1.enter_context(tc.tile_pool(name="pdn", bufs=2))
        pbc = s1.enter_context(tc.tile_pool(name="pbc", bufs=1))
        psA = s1.enter_context(tc.tile_pool(name="psA", bufs=2, space="PSUM"))
        psB = s1.enter_context(tc.tile_pool(name="psB", bufs=2, space="PSUM"))

        # ---- consolidated input tiles ----
        # x^T: one [128, 6*1024] tile, two DMAs (3 chunks each per queue)
        xtall = pbig.tile([128, KC * N], BF16, tag="xtall", name="xtall")
        xtt = [xtall[:, c * N : (c + 1) * N] for c in range(KC)]
        # qk weights: 6 group tiles, one per head-pair; group r holds the
        # q-chunk r (cols 0:768) and k-chunk KC+r (cols 768:1536), each
        # internally (c e)-ordered for [128,128] stationary slices
        wqg = [
            pbig.tile([128, 2 * D], BF16, tag=f"wqg{r}", name=f"wqg{r}")
            for r in range(KC)
        ]
        def load_wqg(r, eng):
            eng.dma_start(wqg[r][:], wqk_d[ts(r, 128), :])

        def wqs(m):
            g, r = divmod(m, KC)
            return wqg[r][:, g * D : (g + 1) * D]

        wvall = pbig.tile([128, KC * D], BF16, tag="wvall", name="wvall")
        wvt = [wvall[:, c * D : (c + 1) * D] for c in range(KC)]
        wmall = pbig.tile([128, KC * D], BF16, tag="wmall", name="wmall")
        wmt = [wmall[:, c * D : (c + 1) * D] for c in range(KC)]

        # ---- startup-critical DMA order ----
        # sync: w_row, x chunks 0-2, wq groups 1,2 ...
        # scalar: wq group 0 (heads 0/1), x chunks 3-5, wv, wq groups 3-5
        # w arrives pre-broadcast from the host ([128, N]) so the gating
        # multiplies never wait on an on-chip partition broadcast
        wb = const.tile([128, N], BF16, tag="wb")
        nc.sync.dma_start(wb[:], w_d[:])
        nc.scalar.dma_start(xtall[:, 0 : 3 * N], xt_d[:, 0 : 3 * N])
        load_wqg(0, nc.sync)
        nc.scalar.dma_start(xtall[:, 3 * N : 6 * N], xt_d[:, 3 * N : 6 * N])
        load_wqg(1, nc.sync)
        nc.scalar.dma_start(wvall[:], wv_d[:])
        load_wqg(2, nc.sync)
        for r in range(3, KC):
            load_wqg(r, nc.scalar)
        # W_msa late on sync (needed only for the output projection)
        nc.sync.dma_start(wmall[:], wm_d[:])

        onescol_f = const.tile([128, HD], F32, tag="onescol_f")
        nc.vector.memset(onescol_f[:], 1.0)
        onescol = const.tile([128, HD], F32R, tag="onescol")
        nc.vector.tensor_copy(onescol[:], onescol_f[:])

        # ---- PE warm-up: the HAM clock gate holds PE at 1.2 GHz until
        # ~3.4us of sustained activity; dependency-free junk matmuls keep
        # the engine running from t~0 while the first DMAs land
        psj = psA.tile([128, 512], F32, tag="psA", name="psj")
        for _ in range(30):
            nc.tensor.matmul(
                psj[0:HD, 0:HD], onescol[:], onescol[:], start=True, stop=True
            )

        # x^T gated once; q, k, v all come out pre-gated downstream
        bias = const.tile([128, KC], F32, tag="bias")
        nc.gpsimd.dma_start(bias[:], bm_d.rearrange("(c p) -> p c", p=128))
        xg = [pbig.tile([128, N], BF16, tag=f"xg{c}", name=f"xg{c}") for c in range(KC)]
        for c in range(KC):
            # bf16 in/out, all SBUF: DVE 4x perf mode (~270ns each)
            nc.vector.tensor_mul(xg[c][:], xtt[c], wb[:])

        qkt = [pqk.tile([128, N], F32R, tag=f"qk{m}", name=f"qk{m}") for m in range(MC_QK)]
        # per-head V block: [1 (ones) | 63 (zero pad) | 64 (v)] = 128 wide.
        # The ones column accumulates the softmax denominator into PSUM
        # partition 0 (where the in-place fast reciprocal may read) while
        # the attention output lands at partitions 64:128 — DVE patterns
        # spanning 64 partitions must start 64-aligned. The constant
        # ones/pad halves of each V tile are initialized once up front.
        VB = 128
        vt = [
            pv.tile([128, H * VB], BF16, tag=f"v{r}", name=f"v{r}")
            for r in range(NT)
        ]
        onespad = const.tile([128, HD], BF16, tag="onespad")
        nc.vector.memset(onespad[:], 0.0)
        nc.vector.memset(onespad[:, 0:1], 1.0)
        for r in range(NT):
            nc.vector.tensor_copy(
                vt[r][:].rearrange("p (h e) -> p h e", e=VB)[:, :, 0:HD],
                onespad[:, None, :].broadcast_to([128, H, HD]),
            )
        ott = [pot.tile([128, N], BF16, tag=f"ot{c}", name=f"ot{c}") for c in range(KC)]
        # output staging: one bf16 buffer, grouped DMAs (host upcasts)
        finall = pbig.tile([128, KC * N], BF16, tag="finall", name="finall")

        def gen_qk(m):
            """qk^T chunk m: [128, N] = W_qkv[:, 128m:...]^T @ xg. Yields per c."""
            wq_m = wqs(m)
            ps = psB.tile([128, N], F32, tag="psB", name="psB")
            for c in range(KC):
                for j in range(NC2):
                    _lab(nc.tensor.matmul(
                        ps[:, ts(j, 512)],
                        wq_m[:, ts(c, 128)],
                        xg[c][:, ts(j, 512)],
                        start=(c == 0),
                        stop=(c == KC - 1),
                    ), f"qk[{m}]c{c}j{j}")
                yield
            # two half-width copies: the second (which frees the PSUM slot
            # for the next head's O' accumulator) is half as long on DVE
            _lab(nc.vector.tensor_copy(qkt[m][:, 0:512], ps[:, 0:512]),
                 f"qkcopy[{m}]j0")
            _lab(nc.vector.tensor_copy(qkt[m][:, 512:1024], ps[:, 512:1024]),
                 f"qkcopy[{m}]j1")
            yield

        def gen_v():
            """V in natural layout + leading ones column per head."""
            for r in range(NT):
                pvp = psB.tile([128, D], F32, tag="psB", name="psB")
                for off, wd in ((0, 512), (512, 256)):
                    for c in range(KC):
                        _lab(nc.tensor.matmul(
                            pvp[:, off : off + wd],
                            xg[c][:, ts(r, 128)],
                            wvt[c][:, off : off + wd],
                            start=(c == 0),
                            stop=(c == KC - 1),
                        ), f"V(r{r},c{c},o{off})")
                v3 = vt[r][:].rearrange("p (h e) -> p h e", e=VB)
                nc.vector.tensor_copy(
                    v3[:, :, HD:VB],
                    pvp[:].rearrange("p (h e) -> p h e", e=HD),
                )
                yield

        def do_o(h, r, e, po):
            # accumulate [1; v]^T @ E^T -> row 0 = softmax denominator,
            # rows 1:65 = unnormalized attention out (transposed); the
            # denominator at PSUM partition 0 lets the fast reciprocal
            # read it in place (custom DVE ops misread PSUM at an offset)
            for j in range(NC2):
                _lab(nc.tensor.matmul(
                    po[:, ts(j, 512)],
                    vt[r][:, h * VB : (h + 1) * VB],
                    e[:, ts(j, 512)],
                    start=(r == 0),
                    stop=False,
                ), f"do_o(h{h},r{r},j{j})")

        def norm_head_j(h, po, j):
            # recip straight off PSUM partition 0; recip/broadcast/multiply
            # per column half, emitted as soon as that half of po is final
            dn = pdn.tile([1, N], F32, tag="dn", name="dn")
            bc = pbc.tile([HD, N], F32, tag="bc", name="bc")
            orow = ott[h // 2][HD * (h % 2) : HD * (h % 2) + HD, :]
            _lab(nc.vector.reciprocal_approx_fast(
                dn[:, ts(j, 512)], po[0:1, ts(j, 512)]
            ), f"recip(h{h},j{j})")
            _lab(nc.gpsimd.partition_broadcast(
                bc[:, ts(j, 512)], dn[:, ts(j, 512)]
            ), f"bcast(h{h},j{j})")
            _lab(nc.vector.tensor_mul(
                orow[:, ts(j, 512)],
                po[HD : 2 * HD, ts(j, 512)],
                bc[:, ts(j, 512)],
            ), f"nmul(h{h},j{j})")

        def gen_attn(h):
            """Attention head h: 8 r-steps, a 4-deep deferred O' drain with
            yields before each drained step (so fillers land ahead of
            instructions gated on the last exp()s), then normalization."""
            qt, qr = qkt[h // 2], HD * (h % 2)
            kt, kr = qkt[KC + h // 2], HD * (h % 2)
            po = psB.tile([VB, N], F32, tag="psB", name="psB")
            pend = []
            for r in range(NT):
                ps = psA.tile([128, N], F32, tag="psA", name="psA")
                for j in range(NC2):
                    _lab(nc.tensor.matmul(
                        ps[:, ts(j, 512)],
                        kt[kr : kr + HD, ts(r, 128)],
                        qt[qr : qr + HD, ts(j, 512)],
                        start=True,
                        stop=True,
                    ), f"S(h{h},r{r},j{j})")
                e = pe_.tile([128, N], BF16, tag="e", name="e")
                _lab(nc.scalar.activation(e[:], ps[:], AF.Exp, scale=SCALE),
                     f"exp(h{h},r{r})")
                if len(pend) == 5:
                    do_o(h, *pend.pop(0), po)
                pend.append((r, e))
                yield
            # drain the five deferred O' steps two-per-yield; the driver
            # interleaves these yields with the NEXT head's first S/exp
            # steps so neither ACT nor PE idles across the head boundary.
            # The final r-step is split by column half with its norm ops
            # emitted immediately after each half completes, so po starts
            # releasing as early as possible.
            for idx in range(4):
                do_o(h, *pend[idx][:2], po)
                if idx % 2 == 1:
                    yield
            r7, e7 = pend[4]
            for j in range(NC2):
                _lab(nc.tensor.matmul(
                    po[:, ts(j, 512)],
                    vt[r7][:, h * VB : (h + 1) * VB],
                    e7[:, ts(j, 512)],
                    start=False,
                    stop=True,
                ), f"do_o(h{h},r7,j{j})")
                norm_head_j(h, po, j)

        def interleave(main, filler, skip=0, ratio=1.5):
            """Exhaust `main`; after main step i >= skip, advance `filler`
            by ~ratio steps (fractional accumulator)."""
            owed = 0.0
            for i, _ in enumerate(main):
                if i >= skip:
                    owed += ratio
                    while owed >= 1.0:
                        next(filler, None)
                        owed -= 1.0
            _run(filler)

        def chain(*gens):
            for g in gens:
                yield from g

        # ---- schedule ----
        # qk chunks for heads 0/1 first (c-steps of chunks 0 and KC zipped
        # so PE consumption tracks the x^T DMA), then V interleaved with
        # head 0; remaining qk chunks ride one-per-head in the attention
        # streams, starting at step 4 so the previous head's PSUM slot and
        # copies are clear before the filler's first allocation.
        g0, g6 = gen_qk(0), gen_qk(KC)
        for _ in range(KC + 1):
            next(g0, None)
            next(g6, None)
        _run(g0)
        _run(g6)

        # head 0 runs with the V projection as its filler; every later
        # head's generator is advanced 8 slots by this driver, with the
        # PREVIOUS head's compressed drain (2 x do_o per yield) and norm
        # overlapped into slots 0-1 and qk fillers from slot 3 (when the
        # psB slot freed by the previous norm is available again)
        cur = gen_attn(0)
        vg = gen_v()
        for i in range(NT):
            next(cur, None)
            next(vg, None)
        _run(vg)
        prev = cur

        filler_map = {
            1: [1, KC + 1, 2],
            2: [KC + 2],
            3: [3],
            4: [KC + 3],
            5: [4],
            6: [KC + 4],
            7: [5],
            8: [KC + 5],
        }
        for h in range(1, H):
            cur = gen_attn(h)
            chunks = filler_map.get(h, [])
            fill = chain(*[gen_qk(m) for m in chunks]) if chunks else None
            steps = len(chunks) * (KC + 1)
            owed = 0.0
            for i in range(NT):
                if prev is not None and i < 2:
                    next(prev, None)
                    if i == 1:
                        _run(prev)
                        prev = None
                next(cur, None)
                if fill is not None and i >= 3:
                    owed += steps / (NT - 3)
                    while owed >= 1.0:
                        next(fill, None)
                        owed -= 1.0
            if fill is not None:
                _run(fill)
            prev = cur
        _run(prev)

        # ---- output projection + bias ----
        # phase 1: chunks 0-3 accumulate k=0..4 in all four PSUM slots
        # (ott[5] is written by head 11's norm, so k=5 is deferred);
        # phase 2: k=5 steps + readouts drain while chunks 4/5 reuse freed
        # slots. Readouts alternate ACT/DVE; output DMAs are grouped.
        def gen_proj(c, ps, k_from, k_to):
            for k in range(k_from, k_to):
                for j in range(NC2):
                    _lab(nc.tensor.matmul(
                        ps[:, ts(j, 512)],
                        wmt[k][:, ts(c, 128)],
                        ott[k][:, ts(j, 512)],
                        start=(k == 0),
                        stop=(k == KC - 1),
                    ), f"proj(c{c},k{k},j{j})")
                yield

        def read_out(c, ps, lo=0, hi=N):
            fin = finall[:, c * N + lo : c * N + hi]
            if c % 2 == 0:
                _lab(nc.scalar.activation(
                    fin, ps[:, lo:hi], AF.Identity, bias=bias[:, c : c + 1]
                ), f"rdout(c{c})")
            else:
                _lab(nc.vector.tensor_scalar_add(
                    fin, ps[:, lo:hi], bias[:, c : c + 1]
                ), f"rdout(c{c})")

        y3 = y_d.rearrange("(c p) q -> p c q", p=128)
        pses = []
        for c in range(4):
            pool = psA if c % 2 == 0 else psB
            ps = pool.tile([128, N], F32, tag="psA" if c % 2 == 0 else "psB",
                           name="psP")
            pses.append(ps)
            _run(gen_proj(c, ps, 0, KC - 1))
        for c in range(4):
            _run(gen_proj(c, pses[c], KC - 1, KC))
            read_out(c, pses[c])
        # first grouped output DMA: chunks 0-2 (sync queue)
        nc.sync.dma_start(
            y3[:, 0:3, :],
            finall[:, 0 : 3 * N].rearrange("p (c q) -> p c q", q=N),
        )
        ps4 = psA.tile([128, N], F32, tag="psA", name="psP")
        _run(gen_proj(4, ps4, 0, KC))
        read_out(4, ps4)
        nc.scalar.dma_start(
            y3[:, 3:5, :],
            finall[:, 3 * N : 5 * N].rearrange("p (c q) -> p c q", q=N),
        )
        # last chunk: halves in separate PSUM tiles; the final transfer is
        # a single [128,512] bf16 piece so the drain tail stays short
        c = KC - 1
        for j in range(NC2):
            pool = psA if j == 0 else psB
            ps = pool.tile([128, N], F32, tag="psA" if j == 0 else "psB",
                           name="psL")
            for k in range(KC):
                _lab(nc.tensor.matmul(
                    ps[:, 0:512],
                    wmt[k][:, ts(c, 128)],
                    ott[k][:, ts(j, 512)],
                    start=(k == 0),
                    stop=(k == KC - 1),
                ), f"projL(j{j},k{k})")
            fin = finall[:, c * N + j * 512 : c * N + (j + 1) * 512]
            if j == 0:
                _lab(nc.scalar.activation(
                    fin, ps[:, 0:512], AF.Identity, bias=bias[:, c : c + 1]
                ), f"rdoutL(j{j})")
            else:
                _lab(nc.vector.tensor_scalar_add(
                    fin, ps[:, 0:512], bias[:, c : c + 1]
                ), f"rdoutL(j{j})")
            eng = nc.sync if j == 0 else nc.scalar
            eng.dma_start(y_d[ts(c, 128), ts(j, 512)], fin)


def _build(repeat=1):
    key = ("nc", repeat)
    if key not in _CACHE:
        nc = bacc.Bacc("TRN2", target_bir_lowering=False, debug=False, num_devices=B)
        with tile.TileContext(nc) as tc:
            _emit(tc, repeat=repeat)
        nc.compile()
        _CACHE[key] = nc
    return _CACHE[key]


def kernel(x, weight, W_qkv, W_msa, b_msa):
    import ml_dtypes

    bf16 = ml_dtypes.bfloat16
    nc = _build()
    x = np.asarray(x, dtype=np.float32)
    weight = np.asarray(weight, dtype=np.float32)
    W_qkv = np.asarray(W_qkv, dtype=np.float32)
    # prepack weights into the on-chip layouts (row-contiguous DMAs):
    # wqk[r]: [128, (g c e)] with m = g*6 + r, source col m*128+e, row c*128+p
    wqk_f = W_qkv[:, : 2 * D].reshape(KC, 128, 2, KC, 128)  # [c,p,g,r,e]
    wqk = np.ascontiguousarray(
        wqk_f.transpose(3, 1, 2, 0, 4).reshape(KC * 128, 2 * D).astype(bf16)
    )  # [r, p, g, c, e] -> rows (r p)
    wv_f = W_qkv[:, 2 * D :].reshape(KC, 128, D)  # [c,p,e]
    wv = np.ascontiguousarray(
        wv_f.transpose(1, 0, 2).reshape(128, KC * D).astype(bf16)
    )
    wm_f = np.asarray(W_msa, dtype=np.float32).reshape(KC, 128, D)
    wm = np.ascontiguousarray(
        wm_f.transpose(1, 0, 2).reshape(128, KC * D).astype(bf16)
    )
    in_maps = []
    for b in range(B):
        xb = x[b].T.reshape(KC, 128, N).transpose(1, 0, 2).reshape(128, KC * N)
        in_maps.append(
            {
                "xt": np.ascontiguousarray(xb.astype(bf16)),
                "w": np.ascontiguousarray(
                    np.broadcast_to(weight[b : b + 1].astype(bf16), (128, N))
                ),
                "wqk": wqk,
                "wv": wv,
                "wmsa": wm,
                "bmsa": np.asarray(b_msa, dtype=np.float32),
            }
        )
    res = run_bass_kernel_spmd(nc, in_maps, list(range(B)))
    out = np.stack(
        [res.results[b]["yt"].astype(np.float32).T for b in range(B)], axis=0
    )
    return np.ascontiguousarray(out)


# revision 52
# speedup vs baseline: 1.0773x; 1.0116x over previous
"""Trainium2 Bass kernel for nn_Attention_35923106463893.

Multi-head attention block:
    qkv = (weight[:, :, None] * (x @ W_qkv)) -> split q,k,v over 12 heads
    A = softmax(q k^T / sqrt(64));  out = (A v) reshaped @ W_msa + b_msa

Sharding: pure data-parallel over batch B=8 -> one batch element per
NeuronCore, no collectives. Host pre-transposes x[b] and pre-casts all
inputs to bf16 (halves DMA bytes; rel-err ~3e-3 vs the 2e-2 gate).

Key structure:
  - gating folded into x^T once (diag(w) (x W) = (diag(w) x) W)
  - attention per head in transposed layout (keys on partitions); V gets
    a leading ones-column so O' row 0 accumulates the softmax
    denominator at PSUM partition 0, where the fast DVE reciprocal can
    read it in place; normalization via Pool partition-broadcast.
  - exp() skips max-subtraction: logits here are in [-7, 7].
  - DMAs are consolidated into few large transfers: the HWDGE queues
    have a ~1.6us per-descriptor-set cadence, so many small DMAs are
    issue-bound, not bandwidth-bound.
  - the in-order PE stream is software-pipelined: qk chunks for later
    heads ride inside the exp-gated attention streams as fillers; the
    output projection defers every ott[5]-dependent k-step past head
    11's normalization and drains through alternating ACT/DVE readouts
    into one bf16 staging buffer written out by 4 grouped DMAs.
"""

from contextlib import ExitStack

import numpy as np

import concourse.bass as bass
import concourse.mybir as mybir
import concourse.tile as tile
from concourse import bacc
from concourse.bass import ts
from concourse.bass_utils import run_bass_kernel_spmd

B, N, D, H = 8, 1024, 768, 12
HD = D // H          # 64
SCALE = HD ** -0.5   # 0.125
KC = D // 128        # 6 contraction chunks
MC_QK = (2 * D) // 128  # 12 row-chunks of [q;k]^T
NT = N // 128        # 8 token chunks
NC2 = N // 512       # 2 moving chunks

F32 = mybir.dt.float32
F32R = mybir.dt.float32r
BF16 = mybir.dt.bfloat16
AF = mybir.ActivationFunctionType

_CACHE: dict = {}
LABELS: dict = {}


def _lab(inst, label):
    try:
        LABELS[inst.ins.name] = label
    except Exception:
        pass
    return inst


def _run(gen):
    for _ in gen:
        pass


def _emit(tc, repeat=1):
    nc = tc.nc
    # all inputs prepacked on the host into the exact SBUF layouts so
    # every load is row-contiguous (the DMA model pays 2x for runs <512B)
    xt_d = nc.dram_tensor("xt", [128, KC * N], BF16, kind="ExternalInput").ap()
    w_d = nc.dram_tensor("w", [128, N], BF16, kind="ExternalInput").ap()
    wqk_d = nc.dram_tensor("wqk", [KC * 128, 2 * D], BF16, kind="ExternalInput").ap()
    wv_d = nc.dram_tensor("wv", [128, KC * D], BF16, kind="ExternalInput").ap()
    wm_d = nc.dram_tensor("wmsa", [128, KC * D], BF16, kind="ExternalInput").ap()
    bm_d = nc.dram_tensor("bmsa", [D], F32, kind="ExternalInput").ap()
    y_d = nc.dram_tensor("yt", [D, N], BF16, kind="ExternalOutput").ap()

    for _rep in range(repeat):
        _emit_body(tc, xt_d, w_d, wqk_d, wv_d, wm_d, bm_d, y_d)


def _emit_body(tc, xt_d, w_d, wqk_d, wv_d, wm_d, bm_d, y_d):
    nc = tc.nc
    with ExitStack() as s1:
        const = s1.enter_context(tc.tile_pool(name="const", bufs=1))
        pbig = s1.enter_context(tc.tile_pool(name="pbig", bufs=1))
        pqk = s1.enter_context(tc.tile_pool(name="pqk", bufs=1))
        pv = s1.enter_context(tc.tile_pool(name="pv", bufs=1))
        pot = s1.enter_context(tc.tile_pool(name="pot", bufs=1))
        pe_ = s1.enter_context(tc.tile_pool(name="pe", bufs=8))
        pdn = s1.enter_context(tc.tile_pool(name="pdn", bufs=2))
        pbc = s1.enter_context(tc.tile_pool(name="pbc", bufs=1))
        psA = s1.enter_context(tc.tile_pool(name="psA", bufs=2, space="PSUM"))
        psB = s1.enter_context(tc.tile_pool(name="psB", bufs=2, space="PSUM"))

        # ---- consolidated input tiles ----
        # x^T: one [128, 6*1024] tile, two DMAs (3 chunks each per queue)
        xtall = pbig.tile([128, KC * N], BF16, tag="xtall", name="xtall")
        xtt = [xtall[:, c * N : (c + 1) * N] for c in range(KC)]
        # qk weights: 6 group tiles, one per head-pair; group r holds the
        # q-chunk r (cols 0:768) and k-chunk KC+r (cols 768:1536), each
        # internally (c e)-ordered for [128,128] stationary slices
        wqg = [
            pbig.tile([128, 2 * D], BF16, tag=f"wqg{r}", name=f"wqg{r}")
            for r in range(KC)
        ]
        def load_wqg(r, eng):
            eng.dma_start(wqg[r][:], wqk_d[ts(r, 128), :])

        def wqs(m):
            g, r = divmod(m, KC)
            return wqg[r][:, g * D : (g + 1) * D]

        wvall = pbig.tile([128, KC * D], BF16, tag="wvall", name="wvall")
        wvt = [wvall[:, c * D : (c + 1) * D] for c in range(KC)]
        wmall = pbig.tile([128, KC * D], BF16, tag="wmall", name="wmall")
        wmt = [wmall[:, c * D : (c + 1) * D] for c in range(KC)]

        # ---- startup-critical DMA order ----
        # sync: w_row, x chunks 0-2, wq groups 1,2 ...
        # scalar: wq group 0 (heads 0/1), x chunks 3-5, wv, wq groups 3-5
        # w arrives pre-broadcast from the host ([128, N]) so the gating
        # multiplies never wait on an on-chip partition broadcast.
        # Transfers serialize on the shared DMA engines, so the startup
        # order feeds the first qk c-steps at their consumption rate:
        # wb, wq group 0, then x chunk-by-chunk.
        wb = const.tile([128, N], BF16, tag="wb")
        nc.scalar.dma_start(xtall[:, 0:N], xt_d[:, 0:N])
        load_wqg(0, nc.sync)
        nc.sync.dma_start(wb[:], w_d[:])
        nc.scalar.dma_start(xtall[:, N : 2 * N], xt_d[:, N : 2 * N])
        for c in range(2, KC):
            eng = nc.sync if c % 2 == 0 else nc.scalar
            eng.dma_start(xtall[:, c * N : (c + 1) * N], xt_d[:, c * N : (c + 1) * N])
        nc.scalar.dma_start(wvall[:], wv_d[:])
        load_wqg(1, nc.sync)
        load_wqg(2, nc.scalar)
        for r in range(3, KC):
            load_wqg(r, nc.scalar)
        # W_msa late on sync (needed only for the output projection)
        nc.sync.dma_start(wmall[:], wm_d[:])

        onescol_f = const.tile([128, HD], F32, tag="onescol_f")
        nc.vector.memset(onescol_f[:], 1.0)
        onescol = const.tile([128, HD], F32R, tag="onescol")
        nc.vector.tensor_copy(onescol[:], onescol_f[:])

        # ---- PE warm-up: the HAM clock gate holds PE at 1.2 GHz until
        # ~3.4us of sustained activity; dependency-free junk matmuls keep
        # the engine running from t~0 while the first DMAs land
        psj = psA.tile([128, 512], F32, tag="psA", name="psj")
        for _ in range(30):
            nc.tensor.matmul(
                psj[0:HD, 0:HD], onescol[:], onescol[:], start=True, stop=True
            )

        # x^T gated once; q, k, v all come out pre-gated downstream
        xg = [pbig.tile([128, N], BF16, tag=f"xg{c}", name=f"xg{c}") for c in range(KC)]
        for c in range(KC):
            # bf16 in/out, all SBUF: DVE 4x perf mode (~270ns each)
            nc.vector.tensor_mul(xg[c][:], xtt[c], wb[:])
        bias = const.tile([128, KC], F32, tag="bias")
        nc.gpsimd.dma_start(bias[:], bm_d.rearrange("(c p) -> p c", p=128))

        qkt = [pqk.tile([128, N], F32R, tag=f"qk{m}", name=f"qk{m}") for m in range(MC_QK)]
        # per-head V block: [1 (ones) | 63 (zero pad) | 64 (v)] = 128 wide.
        # The ones column accumulates the softmax denominator into PSUM
        # partition 0 (where the in-place fast reciprocal may read) while
        # the attention output lands at partitions 64:128 — DVE patterns
        # spanning 64 partitions must start 64-aligned. The constant
        # ones/pad halves of each V tile are initialized once up front.
        VB = 128
        vt = [
            pv.tile([128, H * VB], BF16, tag=f"v{r}", name=f"v{r}")
            for r in range(NT)
        ]
        onespad = const.tile([128, HD], BF16, tag="onespad")
        nc.vector.memset(onespad[:], 0.0)
        nc.vector.memset(onespad[:, 0:1], 1.0)
        for r in range(NT):
            nc.vector.tensor_copy(
                vt[r][:].rearrange("p (h e) -> p h e", e=VB)[:, :, 0:HD],
                onespad[:, None, :].broadcast_to([128, H, HD]),
            )
        ott = [pot.tile([128, N], BF16, tag=f"ot{c}", name=f"ot{c}") for c in range(KC)]
        # output staging: one bf16 buffer, grouped DMAs (host upcasts)
        finall = pbig.tile([128, KC * N], BF16, tag="finall", name="finall")

        def gen_qk(m, copy_eng=None, raw=False):
            """qk^T chunk m: [128, N] = W_qkv[:, 128m:...]^T @ xg. Yields per c.

            With raw=True the matmuls read ungated x^T and the gate rides
            the PSUM->SBUF copies instead — keeps wb off the startup
            critical path for the first two chunks."""
            ceng = copy_eng or nc.vector
            wq_m = wqs(m)
            src_x = xtt if raw else [t[:] for t in xg]
            ps = psB.tile([128, N], F32, tag="psB", name="psB")
            for c in range(KC):
                for j in range(NC2):
                    _lab(nc.tensor.matmul(
                        ps[:, ts(j, 512)],
                        wq_m[:, ts(c, 128)],
                        src_x[c][ts(j, 512)] if False else src_x[c][:, ts(j, 512)],
                        start=(c == 0),
                        stop=(c == KC - 1),
                    ), f"qk[{m}]c{c}j{j}")
                yield
            # two half-width copies: the second (which frees the PSUM slot
            # for the next head's O' accumulator) is half as long on DVE
            if raw:
                _lab(ceng.tensor_mul(qkt[m][:, 0:512], ps[:, 0:512],
                                     wb[:, 0:512]), f"qkcopy[{m}]j0")
                _lab(ceng.tensor_mul(qkt[m][:, 512:1024], ps[:, 512:1024],
                                     wb[:, 512:1024]), f"qkcopy[{m}]j1")
            else:
                _lab(ceng.tensor_copy(qkt[m][:, 0:512], ps[:, 0:512]),
                     f"qkcopy[{m}]j0")
                _lab(ceng.tensor_copy(qkt[m][:, 512:1024], ps[:, 512:1024]),
                     f"qkcopy[{m}]j1")
            yield

        def gen_v():
            """V in natural layout + leading ones column per head."""
            for r in range(NT):
                pvp = psB.tile([128, D], F32, tag="psB", name="psB")
                for off, wd in ((0, 512), (512, 256)):
                    for c in range(KC):
                        _lab(nc.tensor.matmul(
                            pvp[:, off : off + wd],
                            xg[c][:, ts(r, 128)],
                            wvt[c][:, off : off + wd],
                            start=(c == 0),
                            stop=(c == KC - 1),
                        ), f"V(r{r},c{c},o{off})")
                v3 = vt[r][:].rearrange("p (h e) -> p h e", e=VB)
                nc.vector.tensor_copy(
                    v3[:, :, HD:VB],
                    pvp[:].rearrange("p (h e) -> p h e", e=HD),
                )
                yield

        def do_o(h, r, e, po):
            # accumulate [1; v]^T @ E^T -> row 0 = softmax denominator,
            # rows 1:65 = unnormalized attention out (transposed); the
            # denominator at PSUM partition 0 lets the fast reciprocal
            # read it in place (custom DVE ops misread PSUM at an offset)
            for j in range(NC2):
                _lab(nc.tensor.matmul(
                    po[:, ts(j, 512)],
                    vt[r][:, h * VB : (h + 1) * VB],
                    e[:, ts(j, 512)],
                    start=(r == 0),
                    stop=False,
                ), f"do_o(h{h},r{r},j{j})")

        def norm_head_j(h, po, j):
            # recip straight off PSUM partition 0; recip/broadcast/multiply
            # per column half, emitted as soon as that half of po is final
            dn = pdn.tile([1, N], F32, tag="dn", name="dn")
            bc = pbc.tile([HD, N], F32, tag="bc", name="bc")
            orow = ott[h // 2][HD * (h % 2) : HD * (h % 2) + HD, :]
            _lab(nc.vector.reciprocal_approx_fast(
                dn[:, ts(j, 512)], po[0:1, ts(j, 512)]
            ), f"recip(h{h},j{j})")
            _lab(nc.gpsimd.partition_broadcast(
                bc[:, ts(j, 512)], dn[:, ts(j, 512)]
            ), f"bcast(h{h},j{j})")
            _lab(nc.vector.tensor_mul(
                orow[:, ts(j, 512)],
                po[HD : 2 * HD, ts(j, 512)],
                bc[:, ts(j, 512)],
            ), f"nmul(h{h},j{j})")

        def gen_attn(h):
            """Attention head h: 8 r-steps, a 4-deep deferred O' drain with
            yields before each drained step (so fillers land ahead of
            instructions gated on the last exp()s), then normalization."""
            qt, qr = qkt[h // 2], HD * (h % 2)
            kt, kr = qkt[KC + h // 2], HD * (h % 2)
            po = psB.tile([VB, N], F32, tag="psB", name="psB")
            pend = []
            for r in range(NT):
                ps = psA.tile([128, N], F32, tag="psA", name="psA")
                for j in range(NC2):
                    _lab(nc.tensor.matmul(
                        ps[:, ts(j, 512)],
                        kt[kr : kr + HD, ts(r, 128)],
                        qt[qr : qr + HD, ts(j, 512)],
                        start=True,
                        stop=True,
                    ), f"S(h{h},r{r},j{j})")
                e = pe_.tile([128, N], BF16, tag="e", name="e")
                _lab(nc.scalar.activation(e[:], ps[:], AF.Exp, scale=SCALE),
                     f"exp(h{h},r{r})")
                if len(pend) == 5:
                    do_o(h, *pend.pop(0), po)
                pend.append((r, e))
                yield
            # drain the five deferred O' steps two-per-yield; the driver
            # interleaves these yields with the NEXT head's first S/exp
            # steps so neither ACT nor PE idles across the head boundary.
            # The final r-step is split by column half with its norm ops
            # emitted immediately after each half completes, so po starts
            # releasing as early as possible.
            for idx in range(4):
                do_o(h, *pend[idx][:2], po)
                if idx % 2 == 1:
                    yield
            r7, e7 = pend[4]
            for j in range(NC2):
                _lab(nc.tensor.matmul(
                    po[:, ts(j, 512)],
                    vt[r7][:, h * VB : (h + 1) * VB],
                    e7[:, ts(j, 512)],
                    start=False,
                    stop=True,
                ), f"do_o(h{h},r7,j{j})")
                norm_head_j(h, po, j)

        def interleave(main, filler, skip=0, ratio=1.5):
            """Exhaust `main`; after main step i >= skip, advance `filler`
            by ~ratio steps (fractional accumulator)."""
            owed = 0.0
            for i, _ in enumerate(main):
                if i >= skip:
                    owed += ratio
                    while owed >= 1.0:
                        next(filler, None)
                        owed -= 1.0
            _run(filler)

        def chain(*gens):
            for g in gens:
                yield from g

        def gen_proj(c, ps, k_from, k_to):
            for k in range(k_from, k_to):
                for j in range(NC2):
                    _lab(nc.tensor.matmul(
                        ps[:, ts(j, 512)],
                        wmt[k][:, ts(c, 128)],
                        ott[k][:, ts(j, 512)],
                        start=(k == 0),
                        stop=(k == KC - 1),
                    ), f"proj(c{c},k{k},j{j})")
                yield

        # ---- schedule ----
        # qk chunks for heads 0/1 first (c-steps of chunks 0 and KC zipped
        # so PE consumption tracks the x^T DMA), then V interleaved with
        # head 0; remaining qk chunks ride one-per-head in the attention
        # streams, starting at step 4 so the previous head's PSUM slot and
        # copies are clear before the filler's first allocation.
        g0, g6 = gen_qk(0, raw=True), gen_qk(KC, raw=True)
        for _ in range(KC + 1):
            next(g0, None)
            next(g6, None)
        _run(g0)
        _run(g6)

        # head 0 runs with the V projection as its filler; every later
        # head's generator is advanced 8 slots by this driver, with the
        # PREVIOUS head's compressed drain (2 x do_o per yield) and norm
        # overlapped into slots 0-1 and qk fillers from slot 3 (when the
        # psB slot freed by the previous norm is available again)
        cur = gen_attn(0)
        vg = gen_v()
        for i in range(NT):
            next(cur, None)
            next(vg, None)
        _run(vg)
        prev = cur

        filler_map = {
            1: [1, KC + 1],
            2: [2],
            3: [KC + 2],
            4: [3],
            5: [KC + 3],
            6: [4],
            7: [KC + 4],
            8: [5],
            9: [KC + 5],
        }
        for h in range(1, H):
            cur = gen_attn(h)
            chunks = filler_map.get(h, [])
            fill = chain(*[gen_qk(m) for m in chunks]) if chunks else None
            steps = len(chunks) * (KC + 1)
            owed = 0.0
            for i in range(NT):
                if prev is not None and i < 2:
                    next(prev, None)
                    if i == 1:
                        _run(prev)
                        prev = None
                next(cur, None)
                if fill is not None and i >= 4:
                    owed += steps / (NT - 4)
                    while owed >= 1.0:
                        next(fill, None)
                        owed -= 1.0
            if fill is not None:
                _run(fill)
            prev = cur
        _run(prev)

        # ---- output projection + bias ----
        # phase 1: chunks 0-3 accumulate k=0..4 in all four PSUM slots
        # (ott[5] is written by head 11's norm, so k=5 is deferred);
        # phase 2: k=5 steps + readouts drain while chunks 4/5 reuse freed
        # slots. Readouts alternate ACT/DVE; output DMAs are grouped.
        def read_out(c, ps, lo=0, hi=N):
            fin = finall[:, c * N + lo : c * N + hi]
            if c % 2 == 0:
                _lab(nc.scalar.activation(
                    fin, ps[:, lo:hi], AF.Identity, bias=bias[:, c : c + 1]
                ), f"rdout(c{c})")
            else:
                _lab(nc.vector.tensor_scalar_add(
                    fin, ps[:, lo:hi], bias[:, c : c + 1]
                ), f"rdout(c{c})")

        y3 = y_d.rearrange("(c p) q -> p c q", p=128)
        pses = []
        for c in range(4):
            pool = psA if c % 2 == 0 else psB
            ps = pool.tile([128, N], F32, tag="psA" if c % 2 == 0 else "psB",
                           name="psP")
            pses.append(ps)
            _run(gen_proj(c, ps, 0, KC - 1))
        for c in range(4):
            _run(gen_proj(c, pses[c], KC - 1, KC))
            read_out(c, pses[c])
        # first grouped output DMA: chunks 0-2 (sync queue)
        nc.sync.dma_start(
            y3[:, 0:3, :],
            finall[:, 0 : 3 * N].rearrange("p (c q) -> p c q", q=N),
        )
        nc.scalar.dma_start(y_d[ts(3, 128), :], finall[:, 3 * N : 4 * N])
        ps4 = psA.tile([128, N], F32, tag="psA", name="psP")
        _run(gen_proj(4, ps4, 0, KC))
        read_out(4, ps4)
        nc.scalar.dma_start(y_d[ts(4, 128), :], finall[:, 4 * N : 5 * N])
        # last chunk: halves in separate PSUM tiles; the final transfer is
        # a single [128,512] bf16 piece so the drain tail stays short
        c = KC - 1
        for j in range(NC2):
            pool = psA if j == 0 else psB
            ps = pool.tile([128, N], F32, tag="psA" if j == 0 else "psB",
                           name="psL")
            for k in range(KC):
                _lab(nc.tensor.matmul(
                    ps[:, 0:512],
                    wmt[k][:, ts(c, 128)],
                    ott[k][:, ts(j, 512)],
                    start=(k == 0),
                    stop=(k == KC - 1),
                ), f"projL(j{j},k{k})")
            fin = finall[:, c * N + j * 512 : c * N + (j + 1) * 512]
            if j == 0:
                _lab(nc.scalar.activation(
                    fin, ps[:, 0:512], AF.Identity, bias=bias[:, c : c + 1]
                ), f"rdoutL(j{j})")
            else:
                _lab(nc.vector.tensor_scalar_add(
                    fin, ps[:, 0:512], bias[:, c : c + 1]
                ), f"rdoutL(j{j})")
            eng = nc.sync if j == 0 else nc.scalar
            eng.dma_start(y_d[ts(c, 128), ts(j, 512)], fin)


def _build(repeat=1):
    key = ("nc", repeat)
    if key not in _CACHE:
        nc = bacc.Bacc("TRN2", target_bir_lowering=False, debug=False, num_devices=B)
        with tile.TileContext(nc) as tc:
            _emit(tc, repeat=repeat)
        nc.compile()
        _CACHE[key] = nc
    return _CACHE[key]


def kernel(x, weight, W_qkv, W_msa, b_msa):
    import ml_dtypes

    bf16 = ml_dtypes.bfloat16
    nc = _build()
    x = np.asarray(x, dtype=np.float32)
    weight = np.asarray(weight, dtype=np.float32)
    W_qkv = np.asarray(W_qkv, dtype=np.float32)
    # prepack weights into the on-chip layouts (row-contiguous DMAs):
    # wqk[r]: [128, (g c e)] with m = g*6 + r, source col m*128+e, row c*128+p
    wqk_f = W_qkv[:, : 2 * D].reshape(KC, 128, 2, KC, 128)  # [c,p,g,r,e]
    wqk = np.ascontiguousarray(
        wqk_f.transpose(3, 1, 2, 0, 4).reshape(KC * 128, 2 * D).astype(bf16)
    )  # [r, p, g, c, e] -> rows (r p)
    wv_f = W_qkv[:, 2 * D :].reshape(KC, 128, D)  # [c,p,e]
    wv = np.ascontiguousarray(
        wv_f.transpose(1, 0, 2).reshape(128, KC * D).astype(bf16)
    )
    wm_f = np.asarray(W_msa, dtype=np.float32).reshape(KC, 128, D)
    wm = np.ascontiguousarray(
        wm_f.transpose(1, 0, 2).reshape(128, KC * D).astype(bf16)
    )
    in_maps = []
    for b in range(B):
        xb = x[b].T.reshape(KC, 128, N).transpose(1, 0, 2).reshape(128, KC * N)
        in_maps.append(
            {
                "xt": np.ascontiguousarray(xb.astype(bf16)),
                "w": np.ascontiguousarray(
                    np.broadcast_to(weight[b : b + 1].astype(bf16), (128, N))
                ),
                "wqk": wqk,
                "wv": wv,
                "wmsa": wm,
                "bmsa": np.asarray(b_msa, dtype=np.float32),
            }
        )
    res = run_bass_kernel_spmd(nc, in_maps, list(range(B)))
    out = np.stack(
        [res.results[b]["yt"].astype(np.float32).T for b in range(B)], axis=0
    )
    return np.ascontiguousarray(out)


# revision 58
# speedup vs baseline: 1.1004x; 1.0214x over previous
"""Trainium2 Bass kernel for nn_Attention_35923106463893.

Multi-head attention block:
    qkv = (weight[:, :, None] * (x @ W_qkv)) -> split q,k,v over 12 heads
    A = softmax(q k^T / sqrt(64));  out = (A v) reshaped @ W_msa + b_msa

Sharding: pure data-parallel over batch B=8 -> one batch element per
NeuronCore, no collectives. Host pre-transposes x[b] and pre-casts all
inputs to bf16 (halves DMA bytes; rel-err ~3e-3 vs the 2e-2 gate).

Key structure:
  - gating folded into x^T once (diag(w) (x W) = (diag(w) x) W)
  - attention per head in transposed layout (keys on partitions); V gets
    a leading ones-column so O' row 0 accumulates the softmax
    denominator at PSUM partition 0, where the fast DVE reciprocal can
    read it in place; normalization via Pool partition-broadcast.
  - exp() skips max-subtraction: logits here are in [-7, 7].
  - DMAs are consolidated into few large transfers: the HWDGE queues
    have a ~1.6us per-descriptor-set cadence, so many small DMAs are
    issue-bound, not bandwidth-bound.
  - the in-order PE stream is software-pipelined: qk chunks for later
    heads ride inside the exp-gated attention streams as fillers; the
    output projection defers every ott[5]-dependent k-step past head
    11's normalization and drains through alternating ACT/DVE readouts
    into one bf16 staging buffer written out by 4 grouped DMAs.
"""

from contextlib import ExitStack

import numpy as np

import concourse.bass as bass
import concourse.mybir as mybir
import concourse.tile as tile
from concourse import bacc
from concourse.bass import ts
from concourse.bass_utils import run_bass_kernel_spmd

B, N, D, H = 8, 1024, 768, 12
HD = D // H          # 64
SCALE = HD ** -0.5   # 0.125
KC = D // 128        # 6 contraction chunks
MC_QK = (2 * D) // 128  # 12 row-chunks of [q;k]^T
NT = N // 128        # 8 token chunks
NC2 = N // 512       # 2 moving chunks

F32 = mybir.dt.float32
F32R = mybir.dt.float32r
BF16 = mybir.dt.bfloat16
AF = mybir.ActivationFunctionType

_CACHE: dict = {}
LABELS: dict = {}


def _lab(inst, label):
    try:
        LABELS[inst.ins.name] = label
    except Exception:
        pass
    return inst


def _run(gen):
    for _ in gen:
        pass


def _emit(tc, repeat=1):
    nc = tc.nc
    # all inputs prepacked on the host into the exact SBUF layouts so
    # every load is row-contiguous (the DMA model pays 2x for runs <512B)
    xt_d = nc.dram_tensor("xt", [128, KC * N], BF16, kind="ExternalInput").ap()
    w_d = nc.dram_tensor("w", [128, N], BF16, kind="ExternalInput").ap()
    wqk_d = nc.dram_tensor("wqk", [KC * 128, 2 * D], BF16, kind="ExternalInput").ap()
    wv_d = nc.dram_tensor("wv", [128, KC * D], BF16, kind="ExternalInput").ap()
    wm_d = nc.dram_tensor("wmsa", [128, KC * D], BF16, kind="ExternalInput").ap()
    bm_d = nc.dram_tensor("bmsa", [D], F32, kind="ExternalInput").ap()
    y_d = nc.dram_tensor("yt", [D, N], BF16, kind="ExternalOutput").ap()

    for _rep in range(repeat):
        _emit_body(tc, xt_d, w_d, wqk_d, wv_d, wm_d, bm_d, y_d)


def _emit_body(tc, xt_d, w_d, wqk_d, wv_d, wm_d, bm_d, y_d):
    nc = tc.nc
    with ExitStack() as s1:
        const = s1.enter_context(tc.tile_pool(name="const", bufs=1))
        pbig = s1.enter_context(tc.tile_pool(name="pbig", bufs=1))
        pqk = s1.enter_context(tc.tile_pool(name="pqk", bufs=1))
        pv = s1.enter_context(tc.tile_pool(name="pv", bufs=1))
        pot = s1.enter_context(tc.tile_pool(name="pot", bufs=1))
        pe_ = s1.enter_context(tc.tile_pool(name="pe", bufs=9))
        pdn = s1.enter_context(tc.tile_pool(name="pdn", bufs=2))
        pbc = s1.enter_context(tc.tile_pool(name="pbc", bufs=1))
        psA = s1.enter_context(tc.tile_pool(name="psA", bufs=2, space="PSUM"))
        psB = s1.enter_context(tc.tile_pool(name="psB", bufs=2, space="PSUM"))

        # ---- consolidated input tiles ----
        # x^T: one [128, 6*1024] tile, two DMAs (3 chunks each per queue)
        xtall = pbig.tile([128, KC * N], BF16, tag="xtall", name="xtall")
        xtt = [xtall[:, c * N : (c + 1) * N] for c in range(KC)]
        # qk weights: 6 group tiles, one per head-pair; group r holds the
        # q-chunk r (cols 0:768) and k-chunk KC+r (cols 768:1536), each
        # internally (c e)-ordered for [128,128] stationary slices
        wqg = [
            pbig.tile([128, 2 * D], BF16, tag=f"wqg{r}", name=f"wqg{r}")
            for r in range(KC)
        ]
        def load_wqg(r, eng):
            eng.dma_start(wqg[r][:], wqk_d[ts(r, 128), :])

        def wqs(m):
            g, r = divmod(m, KC)
            return wqg[r][:, g * D : (g + 1) * D]

        wvall = pbig.tile([128, KC * D], BF16, tag="wvall", name="wvall")
        wvt = [wvall[:, c * D : (c + 1) * D] for c in range(KC)]
        wmall = pbig.tile([128, KC * D], BF16, tag="wmall", name="wmall")
        wmt = [wmall[:, c * D : (c + 1) * D] for c in range(KC)]

        # ---- startup-critical DMA order ----
        # sync: w_row, x chunks 0-2, wq groups 1,2 ...
        # scalar: wq group 0 (heads 0/1), x chunks 3-5, wv, wq groups 3-5
        # w arrives pre-broadcast from the host ([128, N]) so the gating
        # multiplies never wait on an on-chip partition broadcast.
        # Transfers serialize on the shared DMA engines, so the startup
        # order feeds the first qk c-steps at their consumption rate:
        # wb, wq group 0, then x chunk-by-chunk.
        wb = const.tile([128, N], BF16, tag="wb")
        nc.scalar.dma_start(xtall[:, 0:N], xt_d[:, 0:N])
        load_wqg(0, nc.sync)
        nc.sync.dma_start(wb[:], w_d[:])
        nc.scalar.dma_start(xtall[:, N : 2 * N], xt_d[:, N : 2 * N])
        for c in range(2, KC):
            eng = nc.sync if c % 2 == 0 else nc.scalar
            eng.dma_start(xtall[:, c * N : (c + 1) * N], xt_d[:, c * N : (c + 1) * N])
        nc.scalar.dma_start(wvall[:], wv_d[:])
        load_wqg(1, nc.sync)
        load_wqg(2, nc.scalar)
        for r in range(3, KC):
            load_wqg(r, nc.scalar)
        # W_msa late on sync (needed only for the output projection)
        nc.sync.dma_start(wmall[:], wm_d[:])

        onescol_f = const.tile([128, HD], F32, tag="onescol_f")
        nc.vector.memset(onescol_f[:], 1.0)
        onescol = const.tile([128, HD], F32R, tag="onescol")
        nc.vector.tensor_copy(onescol[:], onescol_f[:])

        # ---- PE warm-up: the HAM clock gate holds PE at 1.2 GHz until
        # ~3.4us of sustained activity; dependency-free junk matmuls keep
        # the engine running from t~0 while the first DMAs land
        psj = psA.tile([128, 512], F32, tag="psA", name="psj")
        for _ in range(30):
            nc.tensor.matmul(
                psj[0:HD, 0:HD], onescol[:], onescol[:], start=True, stop=True
            )

        # x^T gated once; q, k, v all come out pre-gated downstream
        xg = [pbig.tile([128, N], BF16, tag=f"xg{c}", name=f"xg{c}") for c in range(KC)]
        for c in range(KC):
            # bf16 in/out, all SBUF: DVE 4x perf mode (~270ns each)
            nc.vector.tensor_mul(xg[c][:], xtt[c], wb[:])
        bias = const.tile([128, KC], F32, tag="bias")
        nc.gpsimd.dma_start(bias[:], bm_d.rearrange("(c p) -> p c", p=128))

        qkt = [pqk.tile([128, N], F32R, tag=f"qk{m}", name=f"qk{m}") for m in range(MC_QK)]
        # per-head V block: [1 (ones) | 63 (zero pad) | 64 (v)] = 128 wide.
        # The ones column accumulates the softmax denominator into PSUM
        # partition 0 (where the in-place fast reciprocal may read) while
        # the attention output lands at partitions 64:128 — DVE patterns
        # spanning 64 partitions must start 64-aligned. The constant
        # ones/pad halves of each V tile are initialized once up front.
        VB = 128
        vt = [
            pv.tile([128, H * VB], BF16, tag=f"v{r}", name=f"v{r}")
            for r in range(NT)
        ]
        onespad = const.tile([128, HD], BF16, tag="onespad")
        nc.vector.memset(onespad[:], 0.0)
        nc.vector.memset(onespad[:, 0:1], 1.0)
        for r in range(NT):
            nc.vector.tensor_copy(
                vt[r][:].rearrange("p (h e) -> p h e", e=VB)[:, :, 0:HD],
                onespad[:, None, :].broadcast_to([128, H, HD]),
            )
        ott = [pot.tile([128, N], BF16, tag=f"ot{c}", name=f"ot{c}") for c in range(KC)]
        # output staging: one bf16 buffer, grouped DMAs (host upcasts)
        finall = pbig.tile([128, KC * N], BF16, tag="finall", name="finall")

        def gen_qk(m, copy_eng=None, raw=False):
            """qk^T chunk m: [128, N] = W_qkv[:, 128m:...]^T @ xg. Yields per c.

            With raw=True the matmuls read ungated x^T and the gate rides
            the PSUM->SBUF copies instead — keeps wb off the startup
            critical path for the first two chunks."""
            ceng = copy_eng or nc.vector
            wq_m = wqs(m)
            src_x = xtt if raw else [t[:] for t in xg]
            ps = psB.tile([128, N], F32, tag="psB", name="psB")
            for c in range(KC):
                for j in range(NC2):
                    _lab(nc.tensor.matmul(
                        ps[:, ts(j, 512)],
                        wq_m[:, ts(c, 128)],
                        src_x[c][ts(j, 512)] if False else src_x[c][:, ts(j, 512)],
                        start=(c == 0),
                        stop=(c == KC - 1),
                    ), f"qk[{m}]c{c}j{j}")
                yield
            # two half-width copies: the second (which frees the PSUM slot
            # for the next head's O' accumulator) is half as long on DVE
            if raw:
                _lab(ceng.tensor_mul(qkt[m][:, 0:512], ps[:, 0:512],
                                     wb[:, 0:512]), f"qkcopy[{m}]j0")
                _lab(ceng.tensor_mul(qkt[m][:, 512:1024], ps[:, 512:1024],
                                     wb[:, 512:1024]), f"qkcopy[{m}]j1")
            else:
                _lab(ceng.tensor_copy(qkt[m][:, 0:512], ps[:, 0:512]),
                     f"qkcopy[{m}]j0")
                _lab(ceng.tensor_copy(qkt[m][:, 512:1024], ps[:, 512:1024]),
                     f"qkcopy[{m}]j1")
            yield

        def gen_v():
            """V in natural layout + leading ones column per head."""
            for r in range(NT):
                pvp = psB.tile([128, D], F32, tag="psB", name="psB")
                for off, wd in ((0, 512), (512, 256)):
                    for c in range(KC):
                        _lab(nc.tensor.matmul(
                            pvp[:, off : off + wd],
                            xg[c][:, ts(r, 128)],
                            wvt[c][:, off : off + wd],
                            start=(c == 0),
                            stop=(c == KC - 1),
                        ), f"V(r{r},c{c},o{off})")
                v3 = vt[r][:].rearrange("p (h e) -> p h e", e=VB)
                nc.vector.tensor_copy(
                    v3[:, :, HD:VB],
                    pvp[:].rearrange("p (h e) -> p h e", e=HD),
                )
                yield

        def do_o(h, r, e, po):
            # accumulate [1; v]^T @ E^T -> row 0 = softmax denominator,
            # rows 1:65 = unnormalized attention out (transposed); the
            # denominator at PSUM partition 0 lets the fast reciprocal
            # read it in place (custom DVE ops misread PSUM at an offset)
            for j in range(NC2):
                _lab(nc.tensor.matmul(
                    po[:, ts(j, 512)],
                    vt[r][:, h * VB : (h + 1) * VB],
                    e[:, ts(j, 512)],
                    start=(r == 0),
                    stop=False,
                ), f"do_o(h{h},r{r},j{j})")

        def norm_head_j(h, po, j):
            # recip straight off PSUM partition 0; recip/broadcast/multiply
            # per column half, emitted as soon as that half of po is final
            dn = pdn.tile([1, N], F32, tag="dn", name="dn")
            bc = pbc.tile([HD, N], F32, tag="bc", name="bc")
            orow = ott[h // 2][HD * (h % 2) : HD * (h % 2) + HD, :]
            _lab(nc.vector.reciprocal_approx_fast(
                dn[:, ts(j, 512)], po[0:1, ts(j, 512)]
            ), f"recip(h{h},j{j})")
            _lab(nc.gpsimd.partition_broadcast(
                bc[:, ts(j, 512)], dn[:, ts(j, 512)]
            ), f"bcast(h{h},j{j})")
            _lab(nc.vector.tensor_mul(
                orow[:, ts(j, 512)],
                po[HD : 2 * HD, ts(j, 512)],
                bc[:, ts(j, 512)],
            ), f"nmul(h{h},j{j})")

        def gen_attn(h):
            """Attention head h: 8 r-steps, a 4-deep deferred O' drain with
            yields before each drained step (so fillers land ahead of
            instructions gated on the last exp()s), then normalization."""
            qt, qr = qkt[h // 2], HD * (h % 2)
            kt, kr = qkt[KC + h // 2], HD * (h % 2)
            po = psB.tile([VB, N], F32, tag="psB", name="psB")
            pend = []
            for r in range(NT):
                ps = psA.tile([128, N], F32, tag="psA", name="psA")
                for j in range(NC2):
                    _lab(nc.tensor.matmul(
                        ps[:, ts(j, 512)],
                        kt[kr : kr + HD, ts(r, 128)],
                        qt[qr : qr + HD, ts(j, 512)],
                        start=True,
                        stop=True,
                    ), f"S(h{h},r{r},j{j})")
                e = pe_.tile([128, N], BF16, tag="e", name="e")
                _lab(nc.scalar.activation(e[:], ps[:], AF.Exp, scale=SCALE),
                     f"exp(h{h},r{r})")
                if len(pend) == 7:
                    do_o(h, *pend.pop(0), po)
                pend.append((r, e))
                yield
            # drain the five deferred O' steps two-per-yield; the driver
            # interleaves these yields with the NEXT head's first S/exp
            # steps so neither ACT nor PE idles across the head boundary.
            # The final r-step is split by column half with its norm ops
            # emitted immediately after each half completes, so po starts
            # releasing as early as possible.
            for idx in range(6):
                do_o(h, *pend[idx][:2], po)
                if idx % 2 == 1:
                    yield
            r7, e7 = pend[6]
            for j in range(NC2):
                _lab(nc.tensor.matmul(
                    po[:, ts(j, 512)],
                    vt[r7][:, h * VB : (h + 1) * VB],
                    e7[:, ts(j, 512)],
                    start=False,
                    stop=True,
                ), f"do_o(h{h},r7,j{j})")
                norm_head_j(h, po, j)

        def interleave(main, filler, skip=0, ratio=1.5):
            """Exhaust `main`; after main step i >= skip, advance `filler`
            by ~ratio steps (fractional accumulator)."""
            owed = 0.0
            for i, _ in enumerate(main):
                if i >= skip:
                    owed += ratio
                    while owed >= 1.0:
                        next(filler, None)
                        owed -= 1.0
            _run(filler)

        def chain(*gens):
            for g in gens:
                yield from g

        def gen_proj(c, ps, k_from, k_to):
            for k in range(k_from, k_to):
                for j in range(NC2):
                    _lab(nc.tensor.matmul(
                        ps[:, ts(j, 512)],
                        wmt[k][:, ts(c, 128)],
                        ott[k][:, ts(j, 512)],
                        start=(k == 0),
                        stop=(k == KC - 1),
                    ), f"proj(c{c},k{k},j{j})")
                yield

        # ---- schedule ----
        # qk chunks for heads 0/1 first (c-steps of chunks 0 and KC zipped
        # so PE consumption tracks the x^T DMA), then V interleaved with
        # head 0; remaining qk chunks ride one-per-head in the attention
        # streams, starting at step 4 so the previous head's PSUM slot and
        # copies are clear before the filler's first allocation.
        g0, g6 = gen_qk(0, raw=True), gen_qk(KC, raw=True)
        for _ in range(KC + 1):
            next(g0, None)
            next(g6, None)
        _run(g0)
        _run(g6)

        # head 0 runs with the V projection as its filler; every later
        # head's generator is advanced 8 slots by this driver, with the
        # PREVIOUS head's compressed drain (2 x do_o per yield) and norm
        # overlapped into slots 0-1 and qk fillers from slot 3 (when the
        # psB slot freed by the previous norm is available again)
        cur = gen_attn(0)
        vg = gen_v()
        for i in range(NT):
            next(cur, None)
            next(vg, None)
        _run(vg)
        prev = cur

        filler_map = {
            1: [1, KC + 1],
            2: [2],
            3: [KC + 2],
            4: [3],
            5: [KC + 3],
            6: [4],
            7: [KC + 4],
            8: [5],
            9: [KC + 5],
        }
        for h in range(1, H):
            cur = gen_attn(h)
            chunks = filler_map.get(h, [])
            fill = chain(*[gen_qk(m) for m in chunks]) if chunks else None
            steps = len(chunks) * (KC + 1)
            owed = 0.0
            for i in range(NT):
                if prev is not None and i < 3:
                    next(prev, None)
                    if i == 2:
                        _run(prev)
                        prev = None
                next(cur, None)
                if fill is not None and i >= 4:
                    owed += steps / (NT - 4)
                    while owed >= 1.0:
                        next(fill, None)
                        owed -= 1.0
            if fill is not None:
                _run(fill)
            prev = cur
        _run(prev)

        # ---- output projection + bias ----
        # phase 1: chunks 0-3 accumulate k=0..4 in all four PSUM slots
        # (ott[5] is written by head 11's norm, so k=5 is deferred);
        # phase 2: k=5 steps + readouts drain while chunks 4/5 reuse freed
        # slots. Readouts alternate ACT/DVE; output DMAs are grouped.
        def read_out(c, ps, lo=0, hi=N):
            fin = finall[:, c * N + lo : c * N + hi]
            if c % 2 == 0:
                _lab(nc.scalar.activation(
                    fin, ps[:, lo:hi], AF.Identity, bias=bias[:, c : c + 1]
                ), f"rdout(c{c})")
            else:
                _lab(nc.vector.tensor_scalar_add(
                    fin, ps[:, lo:hi], bias[:, c : c + 1]
                ), f"rdout(c{c})")

        y3 = y_d.rearrange("(c p) q -> p c q", p=128)
        pses = []
        for c in range(4):
            pool = psA if c % 2 == 0 else psB
            ps = pool.tile([128, N], F32, tag="psA" if c % 2 == 0 else "psB",
                           name="psP")
            pses.append(ps)
            _run(gen_proj(c, ps, 0, KC - 1))
        for c in range(4):
            _run(gen_proj(c, pses[c], KC - 1, KC))
            read_out(c, pses[c])
        # first grouped output DMA: chunks 0-2 (sync queue)
        nc.sync.dma_start(
            y3[:, 0:3, :],
            finall[:, 0 : 3 * N].rearrange("p (c q) -> p c q", q=N),
        )
        nc.scalar.dma_start(y_d[ts(3, 128), :], finall[:, 3 * N : 4 * N])
        ps4 = psA.tile([128, N], F32, tag="psA", name="psP")
        _run(gen_proj(4, ps4, 0, KC))
        read_out(4, ps4)
        nc.scalar.dma_start(y_d[ts(4, 128), :], finall[:, 4 * N : 5 * N])
        # last chunk: halves in separate PSUM tiles; the final transfer is
        # a single [128,512] bf16 piece so the drain tail stays short
        c = KC - 1
        for j in range(NC2):
            pool = psA if j == 0 else psB
            ps = pool.tile([128, N], F32, tag="psA" if j == 0 else "psB",
                           name="psL")
            for k in range(KC):
                _lab(nc.tensor.matmul(
                    ps[:, 0:512],
                    wmt[k][:, ts(c, 128)],
                    ott[k][:, ts(j, 512)],
                    start=(k == 0),
                    stop=(k == KC - 1),
                ), f"projL(j{j},k{k})")
            fin = finall[:, c * N + j * 512 : c * N + (j + 1) * 512]
            if j == 0:
                _lab(nc.scalar.activation(
                    fin, ps[:, 0:512], AF.Identity, bias=bias[:, c : c + 1]
                ), f"rdoutL(j{j})")
            else:
                _lab(nc.vector.tensor_scalar_add(
                    fin, ps[:, 0:512], bias[:, c : c + 1]
                ), f"rdoutL(j{j})")
            eng = nc.sync if j == 0 else nc.scalar
            eng.dma_start(y_d[ts(c, 128), ts(j, 512)], fin)


def _build(repeat=1):
    key = ("nc", repeat)
    if key not in _CACHE:
        nc = bacc.Bacc("TRN2", target_bir_lowering=False, debug=False, num_devices=B)
        with tile.TileContext(nc) as tc:
            _emit(tc, repeat=repeat)
        nc.compile()
        _CACHE[key] = nc
    return _CACHE[key]


def kernel(x, weight, W_qkv, W_msa, b_msa):
    import ml_dtypes

    bf16 = ml_dtypes.bfloat16
    nc = _build()
    x = np.asarray(x, dtype=np.float32)
    weight = np.asarray(weight, dtype=np.float32)
    W_qkv = np.asarray(W_qkv, dtype=np.float32)
    # prepack weights into the on-chip layouts (row-contiguous DMAs):
    # wqk[r]: [128, (g c e)] with m = g*6 + r, source col m*128+e, row c*128+p
    wqk_f = W_qkv[:, : 2 * D].reshape(KC, 128, 2, KC, 128)  # [c,p,g,r,e]
    wqk = np.ascontiguousarray(
        wqk_f.transpose(3, 1, 2, 0, 4).reshape(KC * 128, 2 * D).astype(bf16)
    )  # [r, p, g, c, e] -> rows (r p)
    wv_f = W_qkv[:, 2 * D :].reshape(KC, 128, D)  # [c,p,e]
    wv = np.ascontiguousarray(
        wv_f.transpose(1, 0, 2).reshape(128, KC * D).astype(bf16)
    )
    wm_f = np.asarray(W_msa, dtype=np.float32).reshape(KC, 128, D)
    wm = np.ascontiguousarray(
        wm_f.transpose(1, 0, 2).reshape(128, KC * D).astype(bf16)
    )
    in_maps = []
    for b in range(B):
        xb = x[b].T.reshape(KC, 128, N).transpose(1, 0, 2).reshape(128, KC * N)
        in_maps.append(
            {
                "xt": np.ascontiguousarray(xb.astype(bf16)),
                "w": np.ascontiguousarray(
                    np.broadcast_to(weight[b : b + 1].astype(bf16), (128, N))
                ),
                "wqk": wqk,
                "wv": wv,
                "wmsa": wm,
                "bmsa": np.asarray(b_msa, dtype=np.float32),
            }
        )
    res = run_bass_kernel_spmd(nc, in_maps, list(range(B)))
    out = np.stack(
        [res.results[b]["yt"].astype(np.float32).T for b in range(B)], axis=0
    )
    return np.ascontiguousarray(out)
